# revision 6
# baseline (speedup 1.0000x reference)
"""ComplEx scoring kernel for Trainium2, sharded over 8 NeuronCores.

Computes: result[b, e] = tmp1[b] . E_im[e] + tmp2[b] . E_re[e] + mask[e]
where tmp1/tmp2 are complex-product combinations of gathered entity and
relation embeddings (with inverse-relation sign handling).

Sharding: entity dimension (100000) split across 8 cores (12500 each).
Batch and relation tables replicated. Each core redundantly computes the
gathered tmp1/tmp2 (needs the full entity table for the gather), then
GEMMs against its own entity shard and emits [1024, 12500] logits.
"""

import sys

sys.path.insert(0, "/opt/trn_rl_repo")

import numpy as np

import concourse.bacc as bacc
import concourse.bass as bass
import concourse.mybir as mybir
import concourse.tile as tile
from concourse.bass import IndirectOffsetOnAxis
from concourse.bass_utils import run_bass_kernel_spmd
from concourse.masks import make_identity

F32 = mybir.dt.float32
F32R = mybir.dt.float32r
I32 = mybir.dt.int32

NUM_ENTITIES = 100000
DIM = 512
BATCH = 1024
NUM_REL = 500  # NUM_REL_TOTAL // 2
N_CORES = 8
ESH = NUM_ENTITIES // N_CORES  # 12500 entities per core
ET = 500                       # entity tile (free dim of each matmul)
N_ET = ESH // ET               # 25 e-tiles per core
NB = BATCH // 128              # 8 batch tiles
NC_D = DIM // 128              # 4 contraction chunks per table


def build_module(use_f32r=True, nrep=1):
    nc = bacc.Bacc("TRN2", target_bir_lowering=False, debug=False)

    hix = nc.dram_tensor("hix", [128, NB], I32, kind="ExternalInput")
    rix = nc.dram_tensor("rix", [128, NB], I32, kind="ExternalInput")
    eim_full = nc.dram_tensor("eim_full", [NUM_ENTITIES, DIM], F32, kind="ExternalInput")
    ere_full = nc.dram_tensor("ere_full", [NUM_ENTITIES, DIM], F32, kind="ExternalInput")
    eimT = nc.dram_tensor("eimT", [DIM, ESH], F32, kind="ExternalInput")
    ereT = nc.dram_tensor("ereT", [DIM, ESH], F32, kind="ExternalInput")
    rim = nc.dram_tensor("rim", [NUM_REL, DIM], F32, kind="ExternalInput")
    rre = nc.dram_tensor("rre", [NUM_REL, DIM], F32, kind="ExternalInput")
    maskrep = nc.dram_tensor("maskrep", [128, ESH], F32, kind="ExternalInput")
    out = nc.dram_tensor("out", [BATCH, ESH], F32, kind="ExternalOutput")

    mm_dt = F32R if use_f32r else F32

    with tile.TileContext(nc) as tc:
        with (
            tc.tile_pool(name="cpool", bufs=1) as cpool,
            tc.tile_pool(name="gpool", bufs=3) as gpool,
            tc.tile_pool(name="epool", bufs=3) as epool,
            tc.tile_pool(name="persist", bufs=1) as ppool,
            tc.tile_pool(name="tps", bufs=4, space="PSUM") as tpsum,
            tc.tile_pool(name="rhspool", bufs=2) as rhspool,
            tc.tile_pool(name="mpool", bufs=2) as mpool,
            tc.tile_pool(name="outpool", bufs=4) as outpool,
            tc.tile_pool(name="psum", bufs=4, space="PSUM") as psum,
        ):
          for _rep in range(nrep):
            # ---- constants / index preprocessing (on device) ----
            identity = cpool.tile([128, 128], F32)
            make_identity(nc, identity[:])

            hix_sb = cpool.tile([128, NB], I32)
            nc.sync.dma_start(hix_sb[:], hix[:])
            rix_sb = cpool.tile([128, NB], I32)
            nc.sync.dma_start(rix_sb[:], rix[:])

            rf = cpool.tile([128, NB], F32)
            nc.vector.tensor_copy(rf[:], rix_sb[:])
            ge = cpool.tile([128, NB], F32)
            nc.vector.tensor_scalar(
                ge[:], rf[:], float(NUM_REL) - 0.5, None, op0=mybir.AluOpType.is_gt
            )
            # sign s = 1 - 2*[r >= NUM_REL]
            sall = cpool.tile([128, NB], F32)
            nc.vector.tensor_scalar(
                sall[:], ge[:], -2.0, 1.0,
                op0=mybir.AluOpType.mult, op1=mybir.AluOpType.add,
            )
            # r_eff = r - NUM_REL*[r >= NUM_REL]
            ge500 = cpool.tile([128, NB], F32)
            nc.vector.tensor_scalar(
                ge500[:], ge[:], float(NUM_REL), None, op0=mybir.AluOpType.mult
            )
            reff_f = cpool.tile([128, NB], F32)
            nc.vector.tensor_sub(reff_f[:], rf[:], ge500[:])
            reff = cpool.tile([128, NB], I32)
            nc.vector.tensor_copy(reff[:], reff_f[:])

            # ---- gather + elementwise + transpose: build tmp1T/tmp2T ----
            # tmp{1,2}T layout: [128 (d within chunk), NB*DIM] where column
            # bt*DIM + c*128 + j holds tmp[bt*128 + j, c*128 + d]
            tmp1T = ppool.tile([128, NB * DIM], mm_dt)
            tmp2T = ppool.tile([128, NB * DIM], mm_dt)

            for bt in range(NB):
                h_im = gpool.tile([128, DIM], F32, tag="h_im")
                nc.gpsimd.indirect_dma_start(
                    out=h_im[:], out_offset=None, in_=eim_full[:],
                    in_offset=IndirectOffsetOnAxis(ap=hix_sb[:, bt : bt + 1], axis=0),
                )
                h_re = gpool.tile([128, DIM], F32, tag="h_re")
                nc.gpsimd.indirect_dma_start(
                    out=h_re[:], out_offset=None, in_=ere_full[:],
                    in_offset=IndirectOffsetOnAxis(ap=hix_sb[:, bt : bt + 1], axis=0),
                )
                r_im = gpool.tile([128, DIM], F32, tag="r_im")
                nc.gpsimd.indirect_dma_start(
                    out=r_im[:], out_offset=None, in_=rim[:],
                    in_offset=IndirectOffsetOnAxis(ap=reff[:, bt : bt + 1], axis=0),
                )
                r_re = gpool.tile([128, DIM], F32, tag="r_re")
                nc.gpsimd.indirect_dma_start(
                    out=r_re[:], out_offset=None, in_=rre[:],
                    in_offset=IndirectOffsetOnAxis(ap=reff[:, bt : bt + 1], axis=0),
                )

                # r_im' = s * r_im  (per-partition scalar)
                rimp = epool.tile([128, DIM], F32, tag="rimp")
                nc.vector.tensor_scalar(
                    rimp[:], r_im[:], sall[:, bt : bt + 1], None,
                    op0=mybir.AluOpType.mult,
                )
                # tmp1 = h_im*r_re + h_re*r_im'
                pa = epool.tile([128, DIM], F32, tag="pa")
                nc.vector.tensor_mul(pa[:], h_im[:], r_re[:])
                pb = epool.tile([128, DIM], F32, tag="pb")
                nc.vector.tensor_mul(pb[:], h_re[:], rimp[:])
                tmp1 = epool.tile([128, DIM], F32, tag="tmp1")
                nc.vector.tensor_add(tmp1[:], pa[:], pb[:])
                # tmp2 = h_re*r_re - h_im*r_im'
                pc = epool.tile([128, DIM], F32, tag="pc")
                nc.vector.tensor_mul(pc[:], h_re[:], r_re[:])
                pd = epool.tile([128, DIM], F32, tag="pd")
                nc.vector.tensor_mul(pd[:], h_im[:], rimp[:])
                tmp2 = epool.tile([128, DIM], F32, tag="tmp2")
                nc.vector.tensor_sub(tmp2[:], pc[:], pd[:])

                for src, dst in ((tmp1, tmp1T), (tmp2, tmp2T)):
                    for c in range(NC_D):
                        pt = tpsum.tile([128, 128], F32, tag="pt")
                        nc.tensor.transpose(
                            pt[:], src[:, c * 128 : (c + 1) * 128], identity[:]
                        )
                        nc.vector.tensor_copy(
                            dst[:, bt * DIM + c * 128 : bt * DIM + (c + 1) * 128],
                            pt[:],
                        )

            # ---- main GEMM: out[b, e] = tmp1 @ E_im^T + tmp2 @ E_re^T + mask ----
            for et in range(N_ET):
                e0 = et * ET
                rhs = rhspool.tile([128, 2 * NC_D * ET], mm_dt, tag="rhs")
                for t, eT in enumerate((eimT, ereT)):
                    for c in range(NC_D):
                        # SWDGE (gpsimd) casts f32 -> f32r during the DMA;
                        # plain HWDGE path when matmuls run in plain f32.
                        dma_eng = nc.gpsimd if use_f32r else nc.sync
                        dma_eng.dma_start(
                            rhs[:, (t * NC_D + c) * ET : (t * NC_D + c + 1) * ET],
                            eT[c * 128 : (c + 1) * 128, e0 : e0 + ET],
                        )
                mtile = mpool.tile([128, ET], F32, tag="mtile")
                nc.sync.dma_start(mtile[:], maskrep[:, e0 : e0 + ET])

                for bt in range(NB):
                    ps = psum.tile([128, ET], F32, tag="ps")
                    k = 0
                    for t, tT in enumerate((tmp1T, tmp2T)):
                        for c in range(NC_D):
                            nc.tensor.matmul(
                                ps[:],
                                lhsT=tT[
                                    :, bt * DIM + c * 128 : bt * DIM + (c + 1) * 128
                                ],
                                rhs=rhs[
                                    :, (t * NC_D + c) * ET : (t * NC_D + c + 1) * ET
                                ],
                                start=(k == 0),
                                stop=(k == 2 * NC_D - 1),
                            )
                            k += 1
                    ot = outpool.tile([128, ET], F32, tag="ot")
                    nc.vector.tensor_add(ot[:], ps[:], mtile[:])
                    nc.sync.dma_start(
                        out[bt * 128 : (bt + 1) * 128, e0 : e0 + ET], ot[:]
                    )

    nc.compile()
    return nc


_NC_CACHE = {}


def _get_module(use_f32r=True):
    key = use_f32r
    if key not in _NC_CACHE:
        _NC_CACHE[key] = build_module(use_f32r)
    return _NC_CACHE[key]


def make_in_maps(h, r, E_im, E_re, R_im, R_re, mask):
    """Host-side sharding / layout prep (value-independent transforms only)."""
    h32 = np.ascontiguousarray(np.asarray(h, dtype=np.int32).reshape(NB, 128).T)
    r32 = np.ascontiguousarray(np.asarray(r, dtype=np.int32).reshape(NB, 128).T)
    E_im = np.asarray(E_im, dtype=np.float32)
    E_re = np.asarray(E_re, dtype=np.float32)
    rim = np.ascontiguousarray(np.asarray(R_im, dtype=np.float32)[:NUM_REL])
    rre = np.ascontiguousarray(np.asarray(R_re, dtype=np.float32)[:NUM_REL])
    mask = np.asarray(mask, dtype=np.float32).reshape(1, NUM_ENTITIES)

    in_maps = []
    for k in range(N_CORES):
        sl = slice(k * ESH, (k + 1) * ESH)
        in_maps.append(
            {
                "hix": h32,
                "rix": r32,
                "eim_full": E_im,
                "ere_full": E_re,
                "eimT": np.ascontiguousarray(E_im[sl].T),
                "ereT": np.ascontiguousarray(E_re[sl].T),
                "rim": rim,
                "rre": rre,
                "maskrep": np.ascontiguousarray(
                    np.broadcast_to(mask[:, sl], (128, ESH))
                ),
            }
        )
    return in_maps


def kernel(h, r, E_im, E_re, R_im, R_re, mask):
    nc = _get_module()
    in_maps = make_in_maps(h, r, E_im, E_re, R_im, R_re, mask)
    res = run_bass_kernel_spmd(nc, in_maps, core_ids=list(range(N_CORES)))
    return np.concatenate([res.results[k]["out"] for k in range(N_CORES)], axis=1)
